# revision 57
# baseline (speedup 1.0000x reference)
"""Trainium2 Bass kernel for nn_Loss_Function_90452011253875.

Detection-style loss: threshold matching (init proposals vs GT lines in
normalized (theta, radius) space), masked regression loss, softmax focal
loss (gamma=2).  Sharding: data-parallel over batch - each of 8 cores
processes 8 images and emits per-partition partial sums; host reduces.

Device algorithm (fp16 on-chip, centered threshold-scaled units):
  The host sorts the 64 batches by valid-GT count and deals them into 8
  slots x 8 cores, so each slot's GT width G_s is the max over its 8
  cores (sum G_s ~ 115-130 vs the padded 8*24 = 192): every grid pass
  shrinks ~1.6x.  Slot order [2nd-smallest, widest..desc, smallest]
  gives a short startup DMA and a short tail.
  Regression is decomposed as (tg-p0)^2 = (dth+e0)^2 with e0 = ti-p0
  host-computed, so only the INIT pairwise planes are ever built:
    per slot: dtq = ct - tr                  (DVE sub, 2x mode)
              theta-test |dth|<1 via Act Abs (DVE Square for the widest
              slots to balance engines), rho-test drh^2<1 via DVE mult
              mx  = max of the two tests     (DVE)
              cond= mx < 1                   (DVE tensor_scalar, 4x)
              m   = dtq * cond               (DVE, masked init diffs)
              PE: G_s ident-matmuls accumulate rhs=[cond|mth|mrh] into a
                  bank-aligned PSUM slot (counts C_n, U_n = sum_g cond*d;
                  full-bank alignment keeps accumulation groups isolated
                  so tail reads are race-free)
              Act: Square(m)+accum (scale folds TH^2*W/2B) = reg main
  Tail (A = slots 0..6 overlap the last PE stream, B = last slot):
    cross terms sum e*U and diag sum e2*C (DVE stt+accum), focal loss
    u = (1-2*(C>0))*d -> sigmoid(u)^2*softplus(u) via sigmoid(-u), Ln
    (a dummy op pre-loads the sigmoid act table off the critical path).
  Per-partition accumulators [P, 24] ship raw; host does final sums.
"""
import os
import sys

for _p in ("/opt/trn_rl_repo", "/root/.axon_site/_ro/trn_rl_repo", "/root/.axon_site"):
    if os.path.isdir(_p) and _p not in sys.path:
        sys.path.append(_p)

import numpy as np

import concourse.bass as bass
import concourse.tile as tile
from concourse import bacc, mybir
from concourse.bass_utils import run_bass_kernel_spmd

F32 = mybir.dt.float32
F16 = mybir.dt.float16
Alu = mybir.AluOpType
Act = mybir.ActivationFunctionType

B, N, G = 64, 16384, 24
NCORES = 8
BPC = B // NCORES          # batches (slots) per core
P = 128
F = N // P
NF = BPC * F

MAX_THETA = 90.0
MAX_RADIUS = 400.0
TH_T = 3.0 / MAX_THETA
TH_R = 20.0 / MAX_RADIUS
W_CLS = 2.0
W_REG = 5.0
PAD = -1000.0
SHIFT = 30000.0
RSC2 = W_REG / (2.0 * B)           # regression scale folded into accums
RSC = float(np.sqrt(RSC2))
FSC = W_CLS / (B * N)              # focal scale folded into the stt accum
CR_T = TH_T * TH_T * W_REG / B     # cross-term fold (into e0 tensor)
CR_R = TH_R * TH_R * W_REG / B

NOUT = 24                          # racc cols: 16 mains, A: 16-19, B: 20-23

_PROGRAMS = {}
_LAST_RESULTS = None


def _build_program(gs):
    """gs: tuple of 8 per-slot GT widths."""
    nc = bacc.Bacc("TRN2", target_bir_lowering=False, debug=False,
                   enable_asserts=False, num_devices=NCORES)

    ct_d = nc.dram_tensor("ct", [BPC, P, 2 * F], F16, kind="ExternalInput").ap()
    tr_d = [nc.dram_tensor("tr%d" % s, [P, 2 * gs[s] * F], F16,
                           kind="ExternalInput").ap() for s in range(BPC)]
    d_d = nc.dram_tensor("d", [P, NF], F16, kind="ExternalInput").ap()
    e0_d = nc.dram_tensor("e0", [P, NF], F16, kind="ExternalInput").ap()
    e1_d = nc.dram_tensor("e1", [P, NF], F16, kind="ExternalInput").ap()
    e2_d = nc.dram_tensor("e2", [P, NF], F16, kind="ExternalInput").ap()
    id_d = nc.dram_tensor("ident", [P, P], F16, kind="ExternalInput").ap()
    out_d = nc.dram_tensor("out", [P, NOUT], F32, kind="ExternalOutput").ap()

    from contextlib import ExitStack
    with tile.TileContext(nc) as tc, ExitStack() as ctx:
        ctp = ctx.enter_context(tc.tile_pool(name="ctp", bufs=3))
        trp = ctx.enter_context(tc.tile_pool(name="trp", bufs=3))
        dtqp = ctx.enter_context(tc.tile_pool(name="dtqp", bufs=2))
        mxp = ctx.enter_context(tc.tile_pool(name="mxp", bufs=2))
        blkp = ctx.enter_context(tc.tile_pool(name="blkp", bufs=3))
        scrp = ctx.enter_context(tc.tile_pool(name="scrp", bufs=1))
        persist = ctx.enter_context(tc.tile_pool(name="persist", bufs=1))
        small = ctx.enter_context(tc.tile_pool(name="small", bufs=1))
        psum = ctx.enter_context(tc.tile_pool(name="psum", bufs=1, space="PSUM"))

        ct_t = [None] * BPC
        tr_t = [None] * BPC
        dtq_t = [None] * BPC
        blk_t = [None] * BPC

        def load(s, split=False):
            g = gs[s]
            tr_t[s] = trp.tile([P, 2 * g * F], F16, name="tr_sb", tag="tr")
            ct_t[s] = ctp.tile([P, 2 * F], F16, name="ct_sb", tag="ct")
            if split:
                nc.sync.dma_start(ct_t[s][:], ct_d[s])
                nc.sync.dma_start(tr_t[s][:, 0:g * F], tr_d[s][:, 0:g * F])
                nc.sync.dma_start(tr_t[s][:, g * F:2 * g * F],
                                  tr_d[s][:, g * F:2 * g * F])
            else:
                nc.sync.dma_start(tr_t[s][:], tr_d[s])
                nc.sync.dma_start(ct_t[s][:], ct_d[s])

        load(0)
        load(1)
        ident = persist.tile([P, P], F16)
        nc.sync.dma_start(ident[:], id_d)
        d_all = persist.tile([P, NF], F16)
        e0_all = persist.tile([P, NF], F16)
        e1_all = persist.tile([P, NF], F16)
        e2_all = persist.tile([P, NF], F16)

        def load_aux():
            nc.sync.dma_start(d_all[:], d_d)
            nc.sync.dma_start(e0_all[:], e0_d)
            nc.sync.dma_start(e1_all[:], e1_d)
            nc.sync.dma_start(e2_all[:], e2_d)

        # PSUM accumulators: per slot [3, F] blocks = (counts, U_theta, U_rho)
        # split A = slots 0..6 / B = slot 7 so the tail's A-half ops only
        # wait on slot 6's last matmul, not the whole stream
        BANK = 512
        cps = psum.tile([P, BPC * BANK], F32)
        cps_b = cps[:].rearrange("p (s x) -> p s x", s=BPC)

        def cps_slot(s):
            return cps_b[:, s, 0:3 * F].rearrange("p (k f) -> p k f", k=3)
        cnt_v = cps_b[:, :, 0:F]
        ut_v = cps_b[:, :, F:2 * F]
        ur_v = cps_b[:, :, 2 * F:3 * F]
        racc = persist.tile([P, NOUT], F32)

        def v3(t):
            return t[:].rearrange("p (s f) -> p s f", s=BPC)

        def sub(s, split=False):
            g = gs[s]
            dtq_t[s] = dtqp.tile([P, 2 * g * F], F16, name="dtq", tag="dtq")
            ct_bc = (ct_t[s][:].rearrange("p (h f) -> p h f", h=2)
                     .unsqueeze(2).broadcast_to([P, 2, g, F]))
            trv = tr_t[s][:].rearrange("p (h g f) -> p h g f", h=2, g=g)
            ov = dtq_t[s][:].rearrange("p (h g f) -> p h g f", h=2, g=g)
            nc.vector.tensor_tensor(ov[:, 0:1], ct_bc[:, 0:1], trv[:, 0:1],
                                    Alu.subtract)
            nc.vector.tensor_tensor(ov[:, 1:2], ct_bc[:, 1:2], trv[:, 1:2],
                                    Alu.subtract)

        qr_t = [None] * BPC
        ab_t = [None] * BPC

        def head(s):
            # |dth|<1 and drh^2<1 are equivalent one-sided tests vs 1;
            # split them DVE/Act so neither engine eats both abs passes.
            # Emitted right after sub(s) so the Act Abs queues ahead of the
            # previous slot's squares and never stalls the DVE max.  The
            # last two slots square on DVE instead: their chain must not
            # wait on Act, which lags near the tail.
            g = gs[s]
            gf = g * F
            dtqv = dtq_t[s][:].rearrange("p (h g f) -> p h g f", h=2, g=g)
            dth = dtqv[:, 0].rearrange("p g f -> p (g f)")
            ab_t[s] = mxp.tile([P, gf], F16, name="ab", tag="ab")
            nc.scalar.activation(ab_t[s][:], dth, Act.Abs)
            qr_t[s] = mxp.tile([P, gf], F16, name="qr", tag="qr")
            drh = dtqv[:, 1].rearrange("p g f -> p (g f)")
            if s in (1, 2, 3):
                # widest slots: square rho on Act to balance engine load
                nc.scalar.activation(qr_t[s][:], drh, Act.Square)
            else:
                nc.vector.tensor_tensor(qr_t[s][:], drh, drh, Alu.mult)

        def tail(s):
            g = gs[s]
            gf = g * F
            dtqv = dtq_t[s][:].rearrange("p (h g f) -> p h g f", h=2, g=g)
            mx = mxp.tile([P, gf], F16, name="mx", tag="mx")
            nc.vector.tensor_tensor(mx[:], ab_t[s][:], qr_t[s][:], Alu.max)
            blk_t[s] = blkp.tile([P, 3 * gf], F16, name="blk", tag="blk")
            bv = blk_t[s][:].rearrange("p (k g f) -> p k g f", k=3, g=g)
            nc.vector.tensor_scalar(blk_t[s][:, 0:gf], mx[:], 1.0, None, Alu.is_lt)
            cond_bc = bv[:, 0:1].broadcast_to([P, 2, g, F])
            nc.vector.tensor_tensor(bv[:, 1:3], dtqv, cond_bc, Alu.mult)
            for gg in range(g):
                nc.tensor.matmul(cps_slot(s), lhsT=ident[:], rhs=bv[:, :, gg],
                                 start=(gg == 0), stop=(gg == g - 1))
            scr = scrp.tile([P, 2 * gf], F16, name="scr", tag="scr")
            nc.scalar.activation(scr[:, 0:gf], blk_t[s][:, gf:2 * gf], Act.Square,
                                 scale=TH_T * RSC,
                                 accum_out=racc[:, 2 * s:2 * s + 1])
            nc.scalar.activation(scr[:, gf:2 * gf], blk_t[s][:, 2 * gf:3 * gf],
                                 Act.Square, scale=TH_R * RSC,
                                 accum_out=racc[:, 2 * s + 1:2 * s + 2])

        # ---- tail: focal + e-terms; A = slots 0..6 (bank-isolated PSUM,
        # safe to read once slot 6's accumulation group stops), B = slot 7.
        # A's whole chain runs under the last slot's matmul stream.
        SPL = (BPC - 1) * F
        z = small.tile([P, NF], F16, name="z", tag="z")
        u = small.tile([P, NF], F16, name="u", tag="u")
        sgneg = small.tile([P, NF], F16, name="sgneg", tag="sgneg")
        lnneg = small.tile([P, NF], F16, name="lnneg", tag="lnneg")
        om = small.tile([P, NF], F16, name="om", tag="om")
        s2 = small.tile([P, NF], F16, name="s2", tag="s2")
        w1 = small.tile([P, NF], F16, name="wx", tag="wx")
        w2 = w1
        w3 = w1
        w4 = w1

        def zu(c0, c1, s0, s1):
            # z = 2 if C<=0 else 0 ; u = (z-1)*d = +d (no match) / -d (match)
            nc.vector.tensor_scalar(
                z[:, c0:c1].rearrange("p (s f) -> p s f", f=F),
                cnt_v[:, s0:s1], 0.0, 2.0, Alu.is_le, Alu.mult)
            nc.vector.scalar_tensor_tensor(u[:, c0:c1], z[:, c0:c1], 1.0,
                                           d_all[:, c0:c1],
                                           Alu.subtract, Alu.mult)

        def eterms(c0, c1, s0, s1, col):
            for w, e, blk0 in ((w1, e0_all, 1), (w2, e1_all, 2), (w3, e2_all, 0)):
                v = cps_b[:, s0:s1, blk0 * F:(blk0 + 1) * F]
                nc.vector.scalar_tensor_tensor(
                    w[:, c0:c1].rearrange("p (s f) -> p s f", f=F),
                    e[:, c0:c1].rearrange("p (s f) -> p s f", f=F), 1.0, v,
                    Alu.mult, Alu.mult, accum_out=racc[:, col:col + 1])
                col += 1

        def om_s2(c0, c1):
            # sigmoid(u) = 1 - sigmoid(-u)
            nc.vector.tensor_scalar(om[:, c0:c1], sgneg[:, c0:c1], -1.0, 1.0,
                                    Alu.mult, Alu.add)
            nc.vector.tensor_tensor(s2[:, c0:c1], om[:, c0:c1], om[:, c0:c1],
                                    Alu.mult)

        def w4acc(c0, c1, col):
            # picked-loss = sigmoid(u)^2 * softplus(u)
            nc.vector.scalar_tensor_tensor(w4[:, c0:c1], lnneg[:, c0:c1], -FSC,
                                           s2[:, c0:c1], Alu.mult, Alu.mult,
                                           accum_out=racc[:, col:col + 1])

        sub(0)
        head(0)
        for s in range(BPC):
            if s + 2 < BPC:
                load(s + 2)
            if s + 1 < BPC:
                sub(s + 1)
                head(s + 1)
            if s == 3:
                load_aux()
            tail(s)
            if s == 5:
                # tiny dummy: forces the sigmoid act-table load to happen
                # here (Act slack) instead of on the critical tail
                dscr = small.tile([P, 2], F16, name="dscr", tag="dscr")
                nc.scalar.activation(dscr[:], d_all[:, 0:2], Act.Sigmoid)

        zu(0, SPL, 0, BPC - 1)
        nc.scalar.activation(sgneg[:, 0:SPL], u[:, 0:SPL], Act.Sigmoid,
                             scale=-1.0)
        eterms(0, SPL, 0, BPC - 1, 16)
        nc.sync.dma_start(out_d[:, 0:16], racc[:, 0:16])
        zu(SPL, NF, BPC - 1, BPC)
        # dummy: pulls the Ln act-table load into Act's idle gap before sigB
        dln = small.tile([P, 2], F16, name="dln", tag="dln")
        nc.scalar.activation(dln[:], sgneg[:, 0:2], Act.Ln)
        nc.scalar.activation(sgneg[:, SPL:NF], u[:, SPL:NF], Act.Sigmoid,
                             scale=-1.0)
        eterms(SPL, NF, BPC - 1, BPC, 20)
        # B's ln first: the B focal chain gates the final racc DMA
        nc.scalar.activation(lnneg[:, SPL:NF], sgneg[:, SPL:NF], Act.Ln)
        nc.scalar.activation(lnneg[:, 0:SPL], sgneg[:, 0:SPL], Act.Ln)
        om_s2(SPL, NF)
        w4acc(SPL, NF, 23)
        om_s2(0, SPL)
        w4acc(0, SPL, 19)
        nc.sync.dma_start(out_d[:, 16:NOUT], racc[:, 16:NOUT])

    nc.compile()
    return nc


def _program_for(gs):
    key = tuple(gs)
    if key not in _PROGRAMS:
        _PROGRAMS[key] = _build_program(key)
    return _PROGRAMS[key]


def _get_program():
    """test.py compatibility: program from the last kernel() call."""
    assert _PROGRAMS, "kernel() must run before profiling"
    return next(iter(_PROGRAMS.values()))


def _host_prep(cls, params, params_init, tgt_params, pts):
    """Slot assignment, threshold-unit scaling, fp16 layouts."""
    cls = np.asarray(cls, dtype=np.float32)
    params = np.asarray(params, dtype=np.float32)
    params_init = np.asarray(params_init, dtype=np.float32)
    tgt_params = np.asarray(tgt_params, dtype=np.float32)
    pts = np.asarray(pts, dtype=np.float32)

    valid = pts[..., 0] != PAD                     # [B, G]
    n_gt = valid.sum(axis=1)                       # [B]
    order = np.argsort(-n_gt, kind="stable")       # batches by GT count desc
    # rank-group r (r=0 widest) holds batches order[r*8:(r+1)*8], one per
    # core; its width is the group max.  Schedule groups into slots as
    # [2nd-smallest, widest ... 3rd-smallest, smallest]: a small slot 0
    # shortens the startup tr-DMA, a small slot 7 shortens the tail.
    perm = [BPC - 2] + list(range(BPC - 2)) + [BPC - 1]
    gs = tuple(int(n_gt[order[perm[s] * NCORES]]) for s in range(BPC))

    ti = params_init[..., 0] / TH_T - 15.0         # [B, N] theta units
    ri = params_init[..., 1] / TH_R - 10.0
    p0 = params[..., 0] / TH_T - 15.0
    p1 = params[..., 1] / TH_R - 10.0
    e0r = ti - p0                                  # raw e in threshold units
    e1r = ri - p1
    e2 = RSC2 * (TH_T * TH_T * e0r * e0r + TH_R * TH_R * e1r * e1r)
    # device dth = ti - tg = -(tg - ti): ship negated e so the cross term
    # sum_n e*U matches 2*sum cond*(tg-ti)*(ti-p0)
    e0 = CR_T * (-e0r)
    e1 = CR_R * (-e1r)
    dd = cls[..., 1] - cls[..., 0]
    t_s = ((tgt_params[..., 0] + MAX_THETA) / (2 * MAX_THETA)) / TH_T - 15.0
    r_s = ((tgt_params[..., 1] + MAX_RADIUS) / (2 * MAX_RADIUS)) / TH_R - 10.0

    ident = np.eye(P, dtype=np.float16)
    in_maps = []
    for c in range(NCORES):
        m = {"ident": ident}
        ct = np.empty((BPC, P, 2 * F), np.float16)
        d_a = np.empty((P, NF), np.float16)
        e0_a = np.empty((P, NF), np.float16)
        e1_a = np.empty((P, NF), np.float16)
        e2_a = np.empty((P, NF), np.float16)
        for s in range(BPC):
            b = int(order[perm[s] * NCORES + c])
            g = gs[s]
            ct[s, :, 0:F] = ti[b].reshape(P, F)
            ct[s, :, F:2 * F] = ri[b].reshape(P, F)
            d_a[:, s * F:(s + 1) * F] = dd[b].reshape(P, F)
            e0_a[:, s * F:(s + 1) * F] = e0[b].reshape(P, F)
            e1_a[:, s * F:(s + 1) * F] = e1[b].reshape(P, F)
            e2_a[:, s * F:(s + 1) * F] = e2[b].reshape(P, F)
            vb = valid[b]
            tg = np.full((2, g), SHIFT, np.float32)
            nv = int(n_gt[b])
            tg[0, :nv] = t_s[b][vb]
            tg[1, :nv] = r_s[b][vb]
            tr = np.broadcast_to(tg.astype(np.float16)[None, :, :, None],
                                 (P, 2, g, F)).reshape(P, 2 * g * F)
            m["tr%d" % s] = np.ascontiguousarray(tr)
        m["ct"] = ct
        m["d"] = d_a
        m["e0"] = e0_a
        m["e1"] = e1_a
        m["e2"] = e2_a
        in_maps.append(m)
    return gs, in_maps


def kernel(cls, params, params_init, tgt_params, pts, profile=False):
    global _LAST_RESULTS
    gs, in_maps = _host_prep(cls, params, params_init, tgt_params, pts)
    nc = _program_for(gs)
    res = run_bass_kernel_spmd(nc, in_maps, list(range(NCORES)), trace=False)
    _LAST_RESULTS = res
    total = np.zeros(2, dtype=np.float64)
    for c in range(NCORES):
        acc = res.results[c]["out"].astype(np.float64)   # [P, NOUT]
        total[0] += acc[:, 19:20].sum() + acc[:, 23:24].sum()
        total[1] += acc[:, 0:19].sum() + acc[:, 20:23].sum()
    return total.astype(np.float32)
